# revision 65
# baseline (speedup 1.0000x reference)
"""Trainium2 Bass kernel for nn_Attention_6743098655482.

Computes, for B=64, H=256, L=8192:
    hidden = concat(sn_hidden, broadcast(mc_hidden))        # [B, 2H, L]
    pre    = tanh(einsum('hk,bkl->bhl', W[0], hidden))      # [B, H, L]
    attns  = einsum('h,bhl->bl', v[0,0], pre)               # [B, L]
    out    = softmax(attns, axis=2)[:, None, :]             # [B, 1, L]

which is equivalent to (per batch b):
    pre_b  = tanh(W1 @ sn_b + (W2 @ mc_b)[:, None])   W1 = W[0][:, :H], W2 = W[0][:, H:]
    out_b  = softmax(v . pre_b)

Sharding: pure data parallel over batch — 8 batches per core on 8 cores,
small params (W, v) replicated. Per core the kernel streams its 64 MB
sn shard from HBM once (memory-bound regime; ~186 us of DMA at 360 GB/s),
so the whole schedule is built to keep that stream gapless and hide
everything else under it.

Dataflow (per core): L is processed in NQ quarter units, batch-inner, so
per (b, q) unit the sn DMA is one [128, QW] transfer per k-half on the SP
queue — nothing else ever issues on that queue, so the stream never stalls
on compute. f32r matmuls run at full PE rate; tanh(+bias) is fused on
ScalarE; the v-dot H-reduction runs as M=1 matmuls LAG chunks behind so
the PE rarely waits on ScalarE. DVE evacuates the v-dot PSUM rows into
per-(group, quarter) staging tiles at partition offsets 0/32/64/96 (the
only offsets compute engines may write). Softmax runs directly on the
staging tiles: exp(+negc shift, accumulating row sums) per (group,
quarter) as soon as its 4 batches finish, reciprocal + scale per group,
and the final 4-row gather DMAs (partition stride 32) write straight to
the output. Each quarter processes group 1's batches first, so group 1's
entire softmax chain hides under the stream and only group 0's last
exp/scale survives into the tail.
"""

import os
import sys

import numpy as np

for _p in ("/opt/trn_rl_repo", "/root/.axon_site/_ro/trn_rl_repo"):
    if os.path.isdir(_p) and _p not in sys.path:
        sys.path.insert(0, _p)

import concourse.bass as bass  # noqa: E402
import concourse.tile as tile  # noqa: E402
from concourse import bacc, mybir  # noqa: E402
from concourse.bass_utils import run_bass_kernel_spmd  # noqa: E402

B, H, L = 64, 256, 8192
NCORES = 8
BL = B // NCORES  # batches per core
F32 = mybir.dt.float32
F32R = mybir.dt.float32r
PK_COLS = 4 * H + 2 * BL + 2  # packed replicated-param layout (see _emit_params)
NEG_COLS = 1 + 132  # negc constant | ind4 [128,4] | ind4T rows (see _emit_params)

CFG = {
    "qw": 2048,  # cols per (b, quarter) unit
    "ch": 1024,  # matmul/activation chunk width
    "seg": 512,  # v-dot segment width (one PSUM bank)
    "lag": 1,  # chunks the att stage trails the pre stage
    "sn_bufs": 4,
    "pre_bufs": 6,
    "ps_pre_bufs": 3,
    "ps_att_bufs": 2,
    "param_dma": "scalar",  # queue for the small replicated-param DMAs
    "b_order": (4, 5, 6, 7, 0, 1, 2, 3),  # group 1 first within each quarter
    "sm_defer": 5,  # chunks between a group's last copy and its exp emission
    "exp_split": 4,  # pieces per (group, quarter) exp
    "exp_spread": 1,  # chunks between successive exp pieces
    "sums_on_dve": 0,  # row-sums via DVE reduce instead of ScalarE accum_out
    "copy_split": 0,  # (unusable: GPSIMD cannot read PSUM on real TRN2)
    "packed_finish": 1,  # densify staging via DMA before exp/scale (32x less ScalarE)
    "pack_w": 64,  # columns per partition in the packed tile
    "repack_lat": 2,  # chunks between repack DMA and its exp
    "exp_defer_extra": 12,  # extra defer for non-tail groups' exp (hides the
    # repack DMA's queueing behind prefetched sn transfers)
    "dma_half": 0,  # batch-outer loop with [128, 2*QW] sn DMAs
    "sn_bufs_half": 5,  # sn ring size in dma_half mode
}


def _emit_params(tc: tile.TileContext, pools, w1t, negc):
    """Load replicated params, compute bias = W2 @ mc, init staging tiles.
    Emitted outside the timing loop."""
    nc = tc.nc
    singles, _, _, ps_pre, _ = pools
    prm = getattr(nc, CFG["param_dma"])
    QW = CFG["qw"]
    NQ = L // QW

    # all replicated params arrive in ONE packed [128, 1042] DMA (pk):
    # w1 k0 | w1 k1 | w2 k0 | w2 k1 | mct k0 | mct k1 | v k0 | v k1
    P = {}
    pk_sb = singles.tile([128, PK_COLS], F32R, tag="pk", name="pk_sb")
    prm.dma_start(out=pk_sb, in_=w1t)
    P["w1"] = [pk_sb[:, 0:H], pk_sb[:, H : 2 * H]]
    P["w2"] = [pk_sb[:, 2 * H : 3 * H], pk_sb[:, 3 * H : 4 * H]]
    P["mct"] = [
        pk_sb[:, 4 * H : 4 * H + BL],
        pk_sb[:, 4 * H + BL : 4 * H + 2 * BL],
    ]
    P["v"] = [
        pk_sb[:, 4 * H + 2 * BL : 4 * H + 2 * BL + 1],
        pk_sb[:, 4 * H + 2 * BL + 1 : 4 * H + 2 * BL + 2],
    ]
    P["negc"] = singles.tile([128, 1], F32, tag="negc", name="negc_sb")
    prm.dma_start(out=P["negc"], in_=negc[:, 0:1])
    if CFG["packed_finish"]:
        ind_sb = singles.tile([128, 132], F32, tag="ind", name="ind_sb")
        prm.dma_start(out=ind_sb, in_=negc[:, 1:133])
        P["ind4"] = ind_sb[:, 0:4]
        P["ind4T"] = ind_sb[0:4, 4:132]

    P["bias"] = []
    for m in range(2):
        bps = ps_pre.tile([128, BL], F32, tag="pspre", name=f"biasps_{m}")
        for k in range(2):
            nc.tensor.matmul(
                bps,
                lhsT=P["w2"][k][:, m * 128 : (m + 1) * 128],
                rhs=P["mct"][k],
                start=(k == 0),
                stop=(k == 1),
            )
        bsb = singles.tile([128, BL], F32, tag=f"bias_{m}", name=f"bias_{m}")
        nc.vector.tensor_copy(out=bsb, in_=bps)
        P["bias"].append(bsb)

    # persistent staging tiles (one per group x quarter); when the packed
    # finish is off they are zero-filled so the inactive partitions always
    # hold finite values for the in-place exp
    P["stg"] = {}
    for g in range(2):
        for q in range(NQ):
            st = singles.tile([128, QW], F32, tag=f"stg_{g}_{q}", name=f"stg_{g}_{q}")
            if not CFG["packed_finish"]:
                nc.gpsimd.memset(st[:], 0.0)
            P["stg"][(g, q)] = st
    if CFG["packed_finish"]:
        W = CFG["pack_w"]
        P["packed"] = {
            g: singles.tile([128, NQ * W], F32, tag=f"pkd_{g}", name=f"pkd_{g}")
            for g in range(2)
        }
        P["qacc"] = [
            singles.tile([128, NQ], F32, tag=f"qacc_{g}", name=f"qacc_{g}")
            for g in range(2)
        ]
    P["sums"] = [
        singles.tile([128, NQ * CFG["exp_split"]], F32, tag=f"sums_{g}", name=f"sums_{g}")
        for g in range(2)
    ]
    P["qsums"] = [
        singles.tile([128, NQ], F32, tag=f"qsums_{g}", name=f"qsums_{g}")
        for g in range(2)
    ]
    P["rec"] = [
        singles.tile([128, 1], F32, tag=f"rec_{g}", name=f"rec_{g}") for g in range(2)
    ]
    return P


def _emit_stream(tc: tile.TileContext, pools, P, sn, out, variant="full"):
    nc = tc.nc
    _, sn_pool, pre_pool, ps_pre, ps_att = pools
    QW = CFG["qw"]
    CH = CFG["ch"]
    SEG = CFG["seg"]
    LAG = CFG["lag"]
    NQ = L // QW
    NCH = QW // CH
    w1_sb, v_sb, bias_sb = P["w1"], P["v"], P["bias"]
    negc_sb = P["negc"]

    pending = []  # deferred att-stage closures (flushed at LAG depth)
    chunk_idx = [0]
    copy_ctr = [0]
    softmax_due = []  # (due chunk index, closure) — deferred so the exp's
    # copy dependencies are satisfied before ScalarE reaches it in order

    def flush_one():
        if pending:
            pending.pop(0)()
        chunk_idx[0] += 1
        while softmax_due and softmax_due[0][0] <= chunk_idx[0]:
            softmax_due.pop(0)[1]()

    tail_g = CFG["b_order"][-1] // 4  # the group whose softmax ends the kernel

    def exp_piece(g, q, e):
        ES = CFG["exp_split"]
        w = QW // ES
        stg = P["stg"][(g, q)]
        if CFG["sums_on_dve"]:
            # keep ScalarE lean: no accumulator read; DVE owns the row sums
            nc.scalar.activation(
                out=stg[:, e * w : (e + 1) * w],
                in_=stg[:, e * w : (e + 1) * w],
                func=mybir.ActivationFunctionType.Exp,
                bias=negc_sb,
            )
        else:
            nc.scalar.activation(
                out=stg[:, e * w : (e + 1) * w],
                in_=stg[:, e * w : (e + 1) * w],
                func=mybir.ActivationFunctionType.Exp,
                bias=negc_sb,
                accum_out=P["sums"][g][:, q * ES + e : q * ES + e + 1],
            )

    def quarter_sum(g, q):
        nc.vector.reduce_sum(
            out=P["qsums"][g][:, q : q + 1],
            in_=P["stg"][(g, q)],
            axis=mybir.AxisListType.X,
        )

    W = CFG["pack_w"]

    def repack(g, q):
        # densify: staging rows 0/32/64/96 x QW cols -> [128, W] packed slice
        # (flat element order matches: p*W+e == b*QW + chunk*W + e)
        nc.gpsimd.dma_start(
            out=P["packed"][g][:, q * W : (q + 1) * W],
            in_=P["stg"][(g, q)][0:97:32, :],
        )

    def exp_packed(g, q):
        pkd = P["packed"][g][:, q * W : (q + 1) * W]
        nc.scalar.activation(
            out=pkd,
            in_=pkd,
            func=mybir.ActivationFunctionType.Exp,
            bias=negc_sb,
            accum_out=P["qacc"][g][:, q : q + 1],
        )

    def finish_packed(g):
        """per-batch totals via indicator matmuls, reciprocal broadcast,
        scale+store of the packed tiles."""
        t128 = pools[0].tile([128, 1], F32, tag=f"t128_{g}", name=f"t128_{g}")
        nc.vector.reduce_sum(out=t128, in_=P["qacc"][g], axis=mybir.AxisListType.X)
        ps4 = ps_att.tile([4, 1], F32, tag="psatt", name=f"ps4_{g}")
        nc.tensor.matmul(ps4, lhsT=P["ind4"], rhs=t128, start=True, stop=True)
        rec4 = pools[0].tile([4, 1], F32, tag=f"rec4_{g}", name=f"rec4_{g}")
        nc.vector.reciprocal(out=rec4, in_=ps4)
        psr = ps_att.tile([128, 1], F32, tag="psatt", name=f"psr_{g}")
        nc.tensor.matmul(psr, lhsT=P["ind4T"], rhs=rec4, start=True, stop=True)
        recp = pools[0].tile([128, 1], F32, tag=f"recp_{g}", name=f"recp_{g}")
        nc.vector.tensor_copy(out=recp, in_=psr)
        pkd = P["packed"][g]
        nc.vector.tensor_scalar_mul(out=pkd, in0=pkd, scalar1=recp)
        for qq in range(NQ):
            if g == tail_g:
                eng = nc.sync if qq % 2 == 0 else nc.scalar
            else:
                eng = nc.gpsimd
            eng.dma_start(
                out=out[4 * g : 4 * g + 4, qq],
                in_=pkd[:, qq * W : (qq + 1) * W],
            )

    def group_finish(g):
        """total sum, reciprocal, scale+store for all quarters of group g."""
        tot = pools[0].tile([128, 1], F32, tag=f"tot_{g}", name=f"tot_{g}")
        src = P["qsums"][g] if CFG["sums_on_dve"] else P["sums"][g]
        nc.vector.reduce_sum(out=tot, in_=src, axis=mybir.AxisListType.X)
        nc.vector.reciprocal(out=P["rec"][g], in_=tot)
        for qq in range(NQ):
            sq = P["stg"][(g, qq)]
            if qq % 2 == 0:
                nc.vector.tensor_scalar_mul(out=sq, in0=sq, scalar1=P["rec"][g])
            else:
                nc.scalar.activation(
                    out=sq,
                    in_=sq,
                    func=mybir.ActivationFunctionType.Copy,
                    scale=P["rec"][g],
                )
            # never on the SP queue mid-stream — the SP DMA ring is FIFO, so a
            # parked output DMA would block later sn transfers. For the tail
            # group the stream is over; alternate queues so DGE gens overlap.
            if g == tail_g:
                eng = nc.sync if qq % 2 == 0 else nc.scalar
            else:
                eng = nc.gpsimd
            eng.dma_start(
                out=out[4 * g : 4 * g + 4, qq * QW : (qq + 1) * QW],
                in_=sq[0:97:32, :],
            )

    def emit_unit(bi, b, q, snt):
        if True:
            g = b // 4
            po = (b % 4) * 32  # staging partition offset
            stg = P["stg"][(g, q)]
            group_done = bi % 4 == 3  # this b completes its group's quarter
            for c in range(NCH):
                col0 = c * CH
                pre_sbs = []
                for m in range(2):
                    pps = ps_pre.tile([128, CH], F32, tag="pspre", name=f"pps_{q}_{b}_{c}_{m}")
                    for s in range(CH // 512):
                        for k in range(2):
                            nc.tensor.matmul(
                                pps[:, s * 512 : (s + 1) * 512],
                                lhsT=w1_sb[k][:, m * 128 : (m + 1) * 128],
                                rhs=snt[k][:, col0 + s * 512 : col0 + (s + 1) * 512],
                                start=(k == 0),
                                stop=(k == 1),
                            )
                    if variant == "mm_only":
                        continue
                    psb = pre_pool.tile([128, CH], F32R, tag="pre", name=f"pre_{q}_{b}_{c}_{m}")
                    nc.scalar.activation(
                        out=psb,
                        in_=pps,
                        func=mybir.ActivationFunctionType.Tanh,
                        bias=bias_sb[m][:, b : b + 1],
                    )
                    pre_sbs.append(psb)
                if variant in ("mm_only", "pre_only"):
                    continue

                last_chunk = c == NCH - 1

                def att_stage(
                    q=q, b=b, g=g, po=po, c=c, col0=col0,
                    stg=stg, pre_sbs=pre_sbs,
                    fin=(last_chunk and group_done and variant == "full"),
                ):
                    for s in range(CH // SEG):
                        aps = ps_att.tile([1, SEG], F32, tag="psatt", name=f"att_{q}_{b}_{c}_{s}")
                        for sp in range(SEG // 512):
                            for m in range(2):
                                nc.tensor.matmul(
                                    aps[:, sp * 512 : (sp + 1) * 512],
                                    lhsT=v_sb[m],
                                    rhs=pre_sbs[m][:, s * SEG + sp * 512 : s * SEG + (sp + 1) * 512],
                                    start=(m == 0),
                                    stop=(m == 1),
                                )
                        copy_ctr[0] += 1
                        ceng = (
                            nc.gpsimd
                            if CFG["copy_split"] and copy_ctr[0] % 2
                            else nc.vector
                        )
                        ceng.tensor_copy(
                            out=stg[po : po + 1, col0 + s * SEG : col0 + (s + 1) * SEG],
                            in_=aps,
                        )
                    if fin:
                        due = chunk_idx[0] + CFG["sm_defer"]
                        if CFG["packed_finish"]:
                            softmax_due.append(
                                (due, lambda g=g, q=q: repack(g, q))
                            )
                            after_exp = due + CFG["repack_lat"]
                            if g != tail_g:
                                after_exp += CFG["exp_defer_extra"]
                            softmax_due.append(
                                (after_exp, lambda g=g, q=q: exp_packed(g, q))
                            )
                            if q == NQ - 1 and not CFG.get("skip_finish"):
                                softmax_due.append(
                                    (after_exp + 1, lambda g=g: finish_packed(g))
                                )
                        else:
                            for e in range(CFG["exp_split"]):
                                softmax_due.append(
                                    (due + e * CFG["exp_spread"],
                                     lambda g=g, q=q, e=e: exp_piece(g, q, e))
                                )
                            after_exp = due + CFG["exp_split"] * CFG["exp_spread"]
                            if CFG["sums_on_dve"]:
                                softmax_due.append(
                                    (after_exp, lambda g=g, q=q: quarter_sum(g, q))
                                )
                                after_exp += 1
                            if q == NQ - 1:
                                softmax_due.append(
                                    (after_exp, lambda g=g: group_finish(g))
                                )

                pending.append(att_stage)
                softmax_due.sort(key=lambda x: x[0])
                if len(pending) > LAG:
                    flush_one()

    if CFG["dma_half"]:
        # batch-outer: one [128, 2*QW] DMA per (b, k, L-half) — half as many,
        # twice-larger descriptors; units consume tile views
        for bi, b in enumerate(CFG["b_order"]):
            snt_half = []
            for q in range(NQ):
                if q % 2 == 0:
                    h = q // 2
                    snt_half = []
                    for k in range(2):
                        t = sn_pool.tile(
                            [128, 2 * QW], F32R, tag="sn", name=f"sn_{b}_{h}_{k}"
                        )
                        nc.sync.dma_start(
                            out=t,
                            in_=sn[
                                b,
                                k * 128 : (k + 1) * 128,
                                h * 2 * QW : (h + 1) * 2 * QW,
                            ],
                        )
                        snt_half.append(t)
                if variant == "dma_only":
                    continue
                snt = [t[:, (q % 2) * QW : (q % 2 + 1) * QW] for t in snt_half]
                emit_unit(bi, b, q, snt)
    else:
        for q in range(NQ):
            for bi, b in enumerate(CFG["b_order"]):
                snt = []
                for k in range(2):
                    t = sn_pool.tile([128, QW], F32R, tag="sn", name=f"sn_{q}_{b}_{k}")
                    nc.sync.dma_start(
                        out=t,
                        in_=sn[b, k * 128 : (k + 1) * 128, q * QW : (q + 1) * QW],
                    )
                    snt.append(t)
                if variant == "dma_only":
                    continue
                emit_unit(bi, b, q, snt)
    while pending or softmax_due:
        flush_one()


def build_module(variant="full", loop_n=None):
    from contextlib import ExitStack

    nc = bacc.Bacc(
        "TRN2",
        debug=False,
        enable_asserts=False,
        target_bir_lowering=False,
    )
    sn = nc.dram_tensor("sn", [BL, H, L], F32R, kind="ExternalInput").ap()
    pk = nc.dram_tensor("pk", [128, PK_COLS], F32R, kind="ExternalInput").ap()
    negc = nc.dram_tensor("negc", [128, NEG_COLS], F32, kind="ExternalInput").ap()
    if CFG["packed_finish"]:
        # same bytes as [BL, L]: row-major [b][q][chunk][W] == [b][q*2048+...]
        NQ = L // CFG["qw"]
        W = CFG["pack_w"]
        out = nc.dram_tensor(
            "out", [BL, NQ, CFG["qw"] // W, W], F32, kind="ExternalOutput"
        ).ap()
    else:
        out = nc.dram_tensor("out", [BL, L], F32, kind="ExternalOutput").ap()
    with tile.TileContext(nc) as tc:
        with ExitStack() as ctx:
            sn_bufs = CFG["sn_bufs_half"] if CFG["dma_half"] else CFG["sn_bufs"]
            pools = (
                ctx.enter_context(tc.tile_pool(name="singles", bufs=1)),
                ctx.enter_context(tc.tile_pool(name="snp", bufs=sn_bufs)),
                ctx.enter_context(tc.tile_pool(name="prep", bufs=CFG["pre_bufs"])),
                ctx.enter_context(
                    tc.tile_pool(name="pspre", bufs=CFG["ps_pre_bufs"], space="PSUM")
                ),
                ctx.enter_context(
                    tc.tile_pool(name="psatt", bufs=CFG["ps_att_bufs"], space="PSUM")
                ),
            )
            P = _emit_params(tc, pools, pk, negc)
            if loop_n is not None:
                with tc.For_i(
                    0,
                    loop_n,
                    1,
                    hint_engines=(
                        mybir.EngineType.PE,
                        mybir.EngineType.Activation,
                        mybir.EngineType.DVE,
                        mybir.EngineType.SP,
                        mybir.EngineType.Pool,
                    ),
                ):
                    _emit_stream(tc, pools, P, sn, out, variant=variant)
            else:
                _emit_stream(tc, pools, P, sn, out, variant=variant)
    nc.compile()
    return nc


_NC = None


def _get_module():
    global _NC
    if _NC is None:
        _NC = build_module()
    return _NC


def make_in_maps(mc_hidden, sn_hidden, v, W):
    """Shard FULL inputs into per-core in_maps (host-side, cheap)."""
    w0 = np.asarray(W, dtype=np.float32)[0]  # [H, 2H]
    w1t = np.ascontiguousarray(w0[:, :H].T)  # [H(k), H(h)]
    w2t = np.ascontiguousarray(w0[:, H:].T)  # [H(k), H(h)]
    vcol = np.ascontiguousarray(np.asarray(v, dtype=np.float32)[0, 0][:, None])
    # upper bound on |attns| = |v . tanh(...)| <= ||v||_1; softmax is invariant
    # to the shift and exp(x - c) stays in fp32 range. Columns 1:133 carry the
    # batch-indicator blocks for the packed softmax finish.
    negc = np.zeros((128, NEG_COLS), dtype=np.float32)
    negc[:, 0] = -np.abs(vcol).sum()
    p_idx = np.arange(128)
    negc[:, 1:5] = (p_idx[:, None] // 32 == np.arange(4)).astype(np.float32)
    negc[0:4, 5:133] = (np.arange(4)[:, None] == p_idx[None, :] // 32).astype(
        np.float32
    )
    mc = np.asarray(mc_hidden, dtype=np.float32)
    sn = np.asarray(sn_hidden, dtype=np.float32)
    in_maps = []
    for c in range(NCORES):
        sl = slice(c * BL, (c + 1) * BL)
        mct = mc[sl].T  # [H, BL]
        pk = np.zeros((128, PK_COLS), dtype=np.float32)
        pk[:, 0:H] = w1t[0:128]
        pk[:, H : 2 * H] = w1t[128:256]
        pk[:, 2 * H : 3 * H] = w2t[0:128]
        pk[:, 3 * H : 4 * H] = w2t[128:256]
        pk[:, 4 * H : 4 * H + BL] = mct[0:128]
        pk[:, 4 * H + BL : 4 * H + 2 * BL] = mct[128:256]
        pk[:, 4 * H + 2 * BL : 4 * H + 2 * BL + 1] = vcol[0:128]
        pk[:, 4 * H + 2 * BL + 1 : 4 * H + 2 * BL + 2] = vcol[128:256]
        in_maps.append(
            {
                "sn": np.ascontiguousarray(sn[sl]),
                "pk": pk,
                "negc": negc,
            }
        )
    return in_maps


def run(mc_hidden, sn_hidden, v, W, trace=False):
    nc = _get_module()
    in_maps = make_in_maps(mc_hidden, sn_hidden, v, W)
    res = run_bass_kernel_spmd(nc, in_maps, core_ids=list(range(NCORES)), trace=False)
    full = np.concatenate(
        [np.asarray(r["out"]).reshape(BL, L) for r in res.results], axis=0
    )
    return full[:, None, :].astype(np.float32), res


def kernel(mc_hidden, sn_hidden, v, W):
    out, _ = run(mc_hidden, sn_hidden, v, W, trace=False)
    return out


# revision 66
# speedup vs baseline: 1.1058x; 1.1058x over previous
"""Trainium2 Bass kernel for nn_Attention_6743098655482.

Computes, for B=64, H=256, L=8192:
    hidden = concat(sn_hidden, broadcast(mc_hidden))        # [B, 2H, L]
    pre    = tanh(einsum('hk,bkl->bhl', W[0], hidden))      # [B, H, L]
    attns  = einsum('h,bhl->bl', v[0,0], pre)               # [B, L]
    out    = softmax(attns, axis=2)[:, None, :]             # [B, 1, L]

which is equivalent to (per batch b):
    pre_b  = tanh(W1 @ sn_b + (W2 @ mc_b)[:, None])   W1 = W[0][:, :H], W2 = W[0][:, H:]
    out_b  = softmax(v . pre_b)

Sharding: pure data parallel over batch — 8 batches per core on 8 cores,
small params (W, v) replicated. Per core the kernel streams its 64 MB
sn shard from HBM once (memory-bound regime; ~186 us of DMA at 360 GB/s),
so the whole schedule is built to keep that stream gapless and hide
everything else under it.

Dataflow (per core): L is processed in NQ quarter units, batch-inner, so
per (b, q) unit the sn DMA is one [128, QW] transfer per k-half on the SP
queue — nothing else ever issues on that queue, so the stream never stalls
on compute (the sn ring is kept shallow so the small auxiliary DMAs don't
queue behind prefetched transfers). f32r matmuls run at full PE rate;
tanh(+bias) is fused on ScalarE; the v-dot H-reduction runs as M=1
matmuls one chunk behind so the PE rarely waits on ScalarE. DVE evacuates
the v-dot PSUM rows into per-(group, quarter) staging tiles at partition
offsets 0/32/64/96 (the only offsets compute engines may write).

Softmax ("packed finish"): a tiny SBUF->SBUF DMA densifies each staging
tile's 4 sparse rows into a [128, 64] slice of the group's packed tile
(all 128 partitions useful), so exp(+negc shift, accumulating row sums)
costs ScalarE 1/32 of the naive width; per-batch totals come from one
[128,4]-indicator matmul, the reciprocal is broadcast back by a second
indicator matmul, one tensor_scalar_mul scales the whole group, and
per-quarter gather DMAs write straight to the output. Each quarter
processes group 1's batches first, so group 1's entire softmax chain
hides under the stream and only group 0's last exp/scale survives into
the tail. exp emission is deferred several chunks so its dependencies are
satisfied before ScalarE reaches it in queue order (the in-order wait
queues are only 4 deep).
"""

import os
import sys

import numpy as np

for _p in ("/opt/trn_rl_repo", "/root/.axon_site/_ro/trn_rl_repo"):
    if os.path.isdir(_p) and _p not in sys.path:
        sys.path.insert(0, _p)

import concourse.bass as bass  # noqa: E402
import concourse.tile as tile  # noqa: E402
from concourse import bacc, mybir  # noqa: E402
from concourse.bass_utils import run_bass_kernel_spmd  # noqa: E402

B, H, L = 64, 256, 8192
NCORES = 8
BL = B // NCORES  # batches per core
F32 = mybir.dt.float32
F32R = mybir.dt.float32r
PK_COLS = 4 * H + 2 * BL + 2  # packed replicated-param layout (see _emit_params)
NEG_COLS = 1 + 132  # negc constant | ind4 [128,4] | ind4T rows (see _emit_params)

CFG = {
    "qw": 2048,  # cols per (b, quarter) unit
    "ch": 1024,  # matmul/activation chunk width
    "seg": 512,  # v-dot segment width (one PSUM bank)
    "lag": 1,  # chunks the att stage trails the pre stage
    "sn_bufs": 4,
    "pre_bufs": 6,
    "ps_pre_bufs": 3,
    "ps_att_bufs": 2,
    "param_dma": "scalar",  # queue for the small replicated-param DMAs
    "b_order": (4, 5, 6, 7, 0, 1, 2, 3),  # group 1 first within each quarter
    "sm_defer": 5,  # chunks between a group's last copy and its exp emission
    "exp_split": 4,  # pieces per (group, quarter) exp
    "exp_spread": 1,  # chunks between successive exp pieces
    "sums_on_dve": 0,  # row-sums via DVE reduce instead of ScalarE accum_out
    "copy_split": 0,  # (unusable: GPSIMD cannot read PSUM on real TRN2)
    "packed_finish": 1,  # densify staging via DMA before exp/scale (32x less ScalarE)
    "pack_w": 64,  # columns per partition in the packed tile
    "repack_lat": 2,  # chunks between repack DMA and its exp
    "exp_defer_extra": 12,  # extra defer for non-tail groups' exp (hides the
    # repack DMA's queueing behind prefetched sn transfers)
    "dma_half": 0,  # batch-outer loop with [128, 2*QW] sn DMAs
    "sn_bufs_half": 5,  # sn ring size in dma_half mode
}


def _emit_params(tc: tile.TileContext, pools, w1t, negc):
    """Load replicated params, compute bias = W2 @ mc, init staging tiles.
    Emitted outside the timing loop."""
    nc = tc.nc
    singles, _, _, ps_pre, _ = pools
    prm = getattr(nc, CFG["param_dma"])
    QW = CFG["qw"]
    NQ = L // QW

    # all replicated params arrive in ONE packed [128, 1042] DMA (pk):
    # w1 k0 | w1 k1 | w2 k0 | w2 k1 | mct k0 | mct k1 | v k0 | v k1
    P = {}
    pk_sb = singles.tile([128, PK_COLS], F32R, tag="pk", name="pk_sb")
    prm.dma_start(out=pk_sb, in_=w1t)
    P["w1"] = [pk_sb[:, 0:H], pk_sb[:, H : 2 * H]]
    P["w2"] = [pk_sb[:, 2 * H : 3 * H], pk_sb[:, 3 * H : 4 * H]]
    P["mct"] = [
        pk_sb[:, 4 * H : 4 * H + BL],
        pk_sb[:, 4 * H + BL : 4 * H + 2 * BL],
    ]
    P["v"] = [
        pk_sb[:, 4 * H + 2 * BL : 4 * H + 2 * BL + 1],
        pk_sb[:, 4 * H + 2 * BL + 1 : 4 * H + 2 * BL + 2],
    ]
    P["negc"] = singles.tile([128, 1], F32, tag="negc", name="negc_sb")
    prm.dma_start(out=P["negc"], in_=negc[:, 0:1])
    if CFG["packed_finish"]:
        ind_sb = singles.tile([128, 132], F32, tag="ind", name="ind_sb")
        prm.dma_start(out=ind_sb, in_=negc[:, 1:133])
        P["ind4"] = ind_sb[:, 0:4]
        P["ind4T"] = ind_sb[0:4, 4:132]

    P["bias"] = []
    for m in range(2):
        bps = ps_pre.tile([128, BL], F32, tag="pspre", name=f"biasps_{m}")
        for k in range(2):
            nc.tensor.matmul(
                bps,
                lhsT=P["w2"][k][:, m * 128 : (m + 1) * 128],
                rhs=P["mct"][k],
                start=(k == 0),
                stop=(k == 1),
            )
        bsb = singles.tile([128, BL], F32, tag=f"bias_{m}", name=f"bias_{m}")
        nc.vector.tensor_copy(out=bsb, in_=bps)
        P["bias"].append(bsb)

    # persistent staging tiles (one per group x quarter); when the packed
    # finish is off they are zero-filled so the inactive partitions always
    # hold finite values for the in-place exp
    P["stg"] = {}
    for g in range(2):
        for q in range(NQ):
            st = singles.tile([128, QW], F32, tag=f"stg_{g}_{q}", name=f"stg_{g}_{q}")
            if not CFG["packed_finish"]:
                nc.gpsimd.memset(st[:], 0.0)
            P["stg"][(g, q)] = st
    if CFG["packed_finish"]:
        W = CFG["pack_w"]
        P["packed"] = {
            g: singles.tile([128, NQ * W], F32, tag=f"pkd_{g}", name=f"pkd_{g}")
            for g in range(2)
        }
        P["qacc"] = [
            singles.tile([128, NQ], F32, tag=f"qacc_{g}", name=f"qacc_{g}")
            for g in range(2)
        ]
    P["sums"] = [
        singles.tile([128, NQ * CFG["exp_split"]], F32, tag=f"sums_{g}", name=f"sums_{g}")
        for g in range(2)
    ]
    P["qsums"] = [
        singles.tile([128, NQ], F32, tag=f"qsums_{g}", name=f"qsums_{g}")
        for g in range(2)
    ]
    P["rec"] = [
        singles.tile([128, 1], F32, tag=f"rec_{g}", name=f"rec_{g}") for g in range(2)
    ]
    return P


def _emit_stream(tc: tile.TileContext, pools, P, sn, out, variant="full"):
    nc = tc.nc
    _, sn_pool, pre_pool, ps_pre, ps_att = pools
    QW = CFG["qw"]
    CH = CFG["ch"]
    SEG = CFG["seg"]
    LAG = CFG["lag"]
    NQ = L // QW
    NCH = QW // CH
    w1_sb, v_sb, bias_sb = P["w1"], P["v"], P["bias"]
    negc_sb = P["negc"]

    pending = []  # deferred att-stage closures (flushed at LAG depth)
    chunk_idx = [0]
    copy_ctr = [0]
    softmax_due = []  # (due chunk index, closure) — deferred so the exp's
    # copy dependencies are satisfied before ScalarE reaches it in order

    def flush_one():
        if pending:
            pending.pop(0)()
        chunk_idx[0] += 1
        while softmax_due and softmax_due[0][0] <= chunk_idx[0]:
            softmax_due.pop(0)[1]()

    tail_g = CFG["b_order"][-1] // 4  # the group whose softmax ends the kernel

    def exp_piece(g, q, e):
        ES = CFG["exp_split"]
        w = QW // ES
        stg = P["stg"][(g, q)]
        if CFG["sums_on_dve"]:
            # keep ScalarE lean: no accumulator read; DVE owns the row sums
            nc.scalar.activation(
                out=stg[:, e * w : (e + 1) * w],
                in_=stg[:, e * w : (e + 1) * w],
                func=mybir.ActivationFunctionType.Exp,
                bias=negc_sb,
            )
        else:
            nc.scalar.activation(
                out=stg[:, e * w : (e + 1) * w],
                in_=stg[:, e * w : (e + 1) * w],
                func=mybir.ActivationFunctionType.Exp,
                bias=negc_sb,
                accum_out=P["sums"][g][:, q * ES + e : q * ES + e + 1],
            )

    def quarter_sum(g, q):
        nc.vector.reduce_sum(
            out=P["qsums"][g][:, q : q + 1],
            in_=P["stg"][(g, q)],
            axis=mybir.AxisListType.X,
        )

    W = CFG["pack_w"]

    def repack(g, q):
        # densify: staging rows 0/32/64/96 x QW cols -> [128, W] packed slice
        # (flat element order matches: p*W+e == b*QW + chunk*W + e)
        nc.gpsimd.dma_start(
            out=P["packed"][g][:, q * W : (q + 1) * W],
            in_=P["stg"][(g, q)][0:97:32, :],
        )

    def exp_packed(g, q):
        pkd = P["packed"][g][:, q * W : (q + 1) * W]
        nc.scalar.activation(
            out=pkd,
            in_=pkd,
            func=mybir.ActivationFunctionType.Exp,
            bias=negc_sb,
            accum_out=P["qacc"][g][:, q : q + 1],
        )

    def finish_packed(g):
        """per-batch totals via indicator matmuls, reciprocal broadcast,
        scale+store of the packed tiles."""
        t128 = pools[0].tile([128, 1], F32, tag=f"t128_{g}", name=f"t128_{g}")
        nc.vector.reduce_sum(out=t128, in_=P["qacc"][g], axis=mybir.AxisListType.X)
        ps4 = ps_att.tile([4, 1], F32, tag="psatt", name=f"ps4_{g}")
        nc.tensor.matmul(ps4, lhsT=P["ind4"], rhs=t128, start=True, stop=True)
        rec4 = pools[0].tile([4, 1], F32, tag=f"rec4_{g}", name=f"rec4_{g}")
        nc.vector.reciprocal(out=rec4, in_=ps4)
        psr = ps_att.tile([128, 1], F32, tag="psatt", name=f"psr_{g}")
        nc.tensor.matmul(psr, lhsT=P["ind4T"], rhs=rec4, start=True, stop=True)
        recp = pools[0].tile([128, 1], F32, tag=f"recp_{g}", name=f"recp_{g}")
        nc.vector.tensor_copy(out=recp, in_=psr)
        pkd = P["packed"][g]
        nc.vector.tensor_scalar_mul(out=pkd, in0=pkd, scalar1=recp)
        for qq in range(NQ):
            if g == tail_g:
                eng = nc.sync if qq % 2 == 0 else nc.scalar
            else:
                eng = nc.gpsimd
            eng.dma_start(
                out=out[4 * g : 4 * g + 4, qq],
                in_=pkd[:, qq * W : (qq + 1) * W],
            )

    def group_finish(g):
        """total sum, reciprocal, scale+store for all quarters of group g."""
        tot = pools[0].tile([128, 1], F32, tag=f"tot_{g}", name=f"tot_{g}")
        src = P["qsums"][g] if CFG["sums_on_dve"] else P["sums"][g]
        nc.vector.reduce_sum(out=tot, in_=src, axis=mybir.AxisListType.X)
        nc.vector.reciprocal(out=P["rec"][g], in_=tot)
        for qq in range(NQ):
            sq = P["stg"][(g, qq)]
            if qq % 2 == 0:
                nc.vector.tensor_scalar_mul(out=sq, in0=sq, scalar1=P["rec"][g])
            else:
                nc.scalar.activation(
                    out=sq,
                    in_=sq,
                    func=mybir.ActivationFunctionType.Copy,
                    scale=P["rec"][g],
                )
            # never on the SP queue mid-stream — the SP DMA ring is FIFO, so a
            # parked output DMA would block later sn transfers. For the tail
            # group the stream is over; alternate queues so DGE gens overlap.
            if g == tail_g:
                eng = nc.sync if qq % 2 == 0 else nc.scalar
            else:
                eng = nc.gpsimd
            eng.dma_start(
                out=out[4 * g : 4 * g + 4, qq * QW : (qq + 1) * QW],
                in_=sq[0:97:32, :],
            )

    def emit_unit(bi, b, q, snt):
        if True:
            g = b // 4
            po = (b % 4) * 32  # staging partition offset
            stg = P["stg"][(g, q)]
            group_done = bi % 4 == 3  # this b completes its group's quarter
            for c in range(NCH):
                col0 = c * CH
                pre_sbs = []
                for m in range(2):
                    pps = ps_pre.tile([128, CH], F32, tag="pspre", name=f"pps_{q}_{b}_{c}_{m}")
                    for s in range(CH // 512):
                        for k in range(2):
                            nc.tensor.matmul(
                                pps[:, s * 512 : (s + 1) * 512],
                                lhsT=w1_sb[k][:, m * 128 : (m + 1) * 128],
                                rhs=snt[k][:, col0 + s * 512 : col0 + (s + 1) * 512],
                                start=(k == 0),
                                stop=(k == 1),
                            )
                    if variant == "mm_only":
                        continue
                    psb = pre_pool.tile([128, CH], F32R, tag="pre", name=f"pre_{q}_{b}_{c}_{m}")
                    nc.scalar.activation(
                        out=psb,
                        in_=pps,
                        func=mybir.ActivationFunctionType.Tanh,
                        bias=bias_sb[m][:, b : b + 1],
                    )
                    pre_sbs.append(psb)
                if variant in ("mm_only", "pre_only"):
                    continue

                last_chunk = c == NCH - 1

                def att_stage(
                    q=q, b=b, g=g, po=po, c=c, col0=col0,
                    stg=stg, pre_sbs=pre_sbs,
                    fin=(last_chunk and group_done and variant == "full"),
                ):
                    for s in range(CH // SEG):
                        aps = ps_att.tile([1, SEG], F32, tag="psatt", name=f"att_{q}_{b}_{c}_{s}")
                        for sp in range(SEG // 512):
                            for m in range(2):
                                nc.tensor.matmul(
                                    aps[:, sp * 512 : (sp + 1) * 512],
                                    lhsT=v_sb[m],
                                    rhs=pre_sbs[m][:, s * SEG + sp * 512 : s * SEG + (sp + 1) * 512],
                                    start=(m == 0),
                                    stop=(m == 1),
                                )
                        copy_ctr[0] += 1
                        ceng = (
                            nc.gpsimd
                            if CFG["copy_split"] and copy_ctr[0] % 2
                            else nc.vector
                        )
                        ceng.tensor_copy(
                            out=stg[po : po + 1, col0 + s * SEG : col0 + (s + 1) * SEG],
                            in_=aps,
                        )
                    if fin:
                        due = chunk_idx[0] + CFG["sm_defer"]
                        if CFG["packed_finish"]:
                            softmax_due.append(
                                (due, lambda g=g, q=q: repack(g, q))
                            )
                            after_exp = due + CFG["repack_lat"]
                            if g != tail_g:
                                after_exp += CFG["exp_defer_extra"]
                            softmax_due.append(
                                (after_exp, lambda g=g, q=q: exp_packed(g, q))
                            )
                            if q == NQ - 1 and not CFG.get("skip_finish"):
                                softmax_due.append(
                                    (after_exp + 1, lambda g=g: finish_packed(g))
                                )
                        else:
                            for e in range(CFG["exp_split"]):
                                softmax_due.append(
                                    (due + e * CFG["exp_spread"],
                                     lambda g=g, q=q, e=e: exp_piece(g, q, e))
                                )
                            after_exp = due + CFG["exp_split"] * CFG["exp_spread"]
                            if CFG["sums_on_dve"]:
                                softmax_due.append(
                                    (after_exp, lambda g=g, q=q: quarter_sum(g, q))
                                )
                                after_exp += 1
                            if q == NQ - 1:
                                softmax_due.append(
                                    (after_exp, lambda g=g: group_finish(g))
                                )

                pending.append(att_stage)
                softmax_due.sort(key=lambda x: x[0])
                if len(pending) > LAG:
                    flush_one()

    if CFG["dma_half"]:
        # batch-outer: one [128, 2*QW] DMA per (b, k, L-half) — half as many,
        # twice-larger descriptors; units consume tile views
        for bi, b in enumerate(CFG["b_order"]):
            snt_half = []
            for q in range(NQ):
                if q % 2 == 0:
                    h = q // 2
                    snt_half = []
                    for k in range(2):
                        t = sn_pool.tile(
                            [128, 2 * QW], F32R, tag="sn", name=f"sn_{b}_{h}_{k}"
                        )
                        nc.sync.dma_start(
                            out=t,
                            in_=sn[
                                b,
                                k * 128 : (k + 1) * 128,
                                h * 2 * QW : (h + 1) * 2 * QW,
                            ],
                        )
                        snt_half.append(t)
                if variant == "dma_only":
                    continue
                snt = [t[:, (q % 2) * QW : (q % 2 + 1) * QW] for t in snt_half]
                emit_unit(bi, b, q, snt)
    else:
        for q in range(NQ):
            for bi, b in enumerate(CFG["b_order"]):
                snt = []
                for k in range(2):
                    t = sn_pool.tile([128, QW], F32R, tag="sn", name=f"sn_{q}_{b}_{k}")
                    nc.sync.dma_start(
                        out=t,
                        in_=sn[b, k * 128 : (k + 1) * 128, q * QW : (q + 1) * QW],
                    )
                    snt.append(t)
                if variant == "dma_only":
                    continue
                emit_unit(bi, b, q, snt)
    while pending or softmax_due:
        flush_one()


def build_module(variant="full", loop_n=None):
    from contextlib import ExitStack

    nc = bacc.Bacc(
        "TRN2",
        debug=False,
        enable_asserts=False,
        target_bir_lowering=False,
    )
    sn = nc.dram_tensor("sn", [BL, H, L], F32R, kind="ExternalInput").ap()
    pk = nc.dram_tensor("pk", [128, PK_COLS], F32R, kind="ExternalInput").ap()
    negc = nc.dram_tensor("negc", [128, NEG_COLS], F32, kind="ExternalInput").ap()
    if CFG["packed_finish"]:
        # same bytes as [BL, L]: row-major [b][q][chunk][W] == [b][q*2048+...]
        NQ = L // CFG["qw"]
        W = CFG["pack_w"]
        out = nc.dram_tensor(
            "out", [BL, NQ, CFG["qw"] // W, W], F32, kind="ExternalOutput"
        ).ap()
    else:
        out = nc.dram_tensor("out", [BL, L], F32, kind="ExternalOutput").ap()
    with tile.TileContext(nc) as tc:
        with ExitStack() as ctx:
            sn_bufs = CFG["sn_bufs_half"] if CFG["dma_half"] else CFG["sn_bufs"]
            pools = (
                ctx.enter_context(tc.tile_pool(name="singles", bufs=1)),
                ctx.enter_context(tc.tile_pool(name="snp", bufs=sn_bufs)),
                ctx.enter_context(tc.tile_pool(name="prep", bufs=CFG["pre_bufs"])),
                ctx.enter_context(
                    tc.tile_pool(name="pspre", bufs=CFG["ps_pre_bufs"], space="PSUM")
                ),
                ctx.enter_context(
                    tc.tile_pool(name="psatt", bufs=CFG["ps_att_bufs"], space="PSUM")
                ),
            )
            P = _emit_params(tc, pools, pk, negc)
            if loop_n is not None:
                with tc.For_i(
                    0,
                    loop_n,
                    1,
                    hint_engines=(
                        mybir.EngineType.PE,
                        mybir.EngineType.Activation,
                        mybir.EngineType.DVE,
                        mybir.EngineType.SP,
                        mybir.EngineType.Pool,
                    ),
                ):
                    _emit_stream(tc, pools, P, sn, out, variant=variant)
            else:
                _emit_stream(tc, pools, P, sn, out, variant=variant)
    nc.compile()
    return nc


_NC = None


def _get_module():
    global _NC
    if _NC is None:
        _NC = build_module()
    return _NC


def make_in_maps(mc_hidden, sn_hidden, v, W):
    """Shard FULL inputs into per-core in_maps (host-side, cheap)."""
    w0 = np.asarray(W, dtype=np.float32)[0]  # [H, 2H]
    w1t = np.ascontiguousarray(w0[:, :H].T)  # [H(k), H(h)]
    w2t = np.ascontiguousarray(w0[:, H:].T)  # [H(k), H(h)]
    vcol = np.ascontiguousarray(np.asarray(v, dtype=np.float32)[0, 0][:, None])
    # upper bound on |attns| = |v . tanh(...)| <= ||v||_1; softmax is invariant
    # to the shift and exp(x - c) stays in fp32 range. Columns 1:133 carry the
    # batch-indicator blocks for the packed softmax finish.
    negc = np.zeros((128, NEG_COLS), dtype=np.float32)
    negc[:, 0] = -np.abs(vcol).sum()
    p_idx = np.arange(128)
    negc[:, 1:5] = (p_idx[:, None] // 32 == np.arange(4)).astype(np.float32)
    negc[0:4, 5:133] = (np.arange(4)[:, None] == p_idx[None, :] // 32).astype(
        np.float32
    )
    mc = np.asarray(mc_hidden, dtype=np.float32)
    sn = np.asarray(sn_hidden, dtype=np.float32)
    in_maps = []
    for c in range(NCORES):
        sl = slice(c * BL, (c + 1) * BL)
        mct = mc[sl].T  # [H, BL]
        pk = np.zeros((128, PK_COLS), dtype=np.float32)
        pk[:, 0:H] = w1t[0:128]
        pk[:, H : 2 * H] = w1t[128:256]
        pk[:, 2 * H : 3 * H] = w2t[0:128]
        pk[:, 3 * H : 4 * H] = w2t[128:256]
        pk[:, 4 * H : 4 * H + BL] = mct[0:128]
        pk[:, 4 * H + BL : 4 * H + 2 * BL] = mct[128:256]
        pk[:, 4 * H + 2 * BL : 4 * H + 2 * BL + 1] = vcol[0:128]
        pk[:, 4 * H + 2 * BL + 1 : 4 * H + 2 * BL + 2] = vcol[128:256]
        in_maps.append(
            {
                "sn": np.ascontiguousarray(sn[sl]),
                "pk": pk,
                "negc": negc,
            }
        )
    return in_maps


def run(mc_hidden, sn_hidden, v, W, trace=False):
    nc = _get_module()
    in_maps = make_in_maps(mc_hidden, sn_hidden, v, W)
    res = run_bass_kernel_spmd(nc, in_maps, core_ids=list(range(NCORES)), trace=False)
    full = np.concatenate(
        [np.asarray(r["out"]).reshape(BL, L) for r in res.results], axis=0
    )
    return full[:, None, :].astype(np.float32), res


def kernel(mc_hidden, sn_hidden, v, W):
    out, _ = run(mc_hidden, sn_hidden, v, W, trace=False)
    return out


# revision 75
# speedup vs baseline: 1.1647x; 1.0532x over previous
"""Trainium2 Bass kernel for nn_Attention_6743098655482.

Computes, for B=64, H=256, L=8192:
    hidden = concat(sn_hidden, broadcast(mc_hidden))        # [B, 2H, L]
    pre    = tanh(einsum('hk,bkl->bhl', W[0], hidden))      # [B, H, L]
    attns  = einsum('h,bhl->bl', v[0,0], pre)               # [B, L]
    out    = softmax(attns, axis=2)[:, None, :]             # [B, 1, L]

which is equivalent to (per batch b):
    pre_b  = tanh(W1 @ sn_b + (W2 @ mc_b)[:, None])   W1 = W[0][:, :H], W2 = W[0][:, H:]
    out_b  = softmax(v . pre_b)

Sharding: pure data parallel over batch — 8 batches per core on 8 cores,
small params (W, v) replicated. Per core the kernel streams its 64 MB
sn shard from HBM once (memory-bound regime; ~186 us of DMA at 360 GB/s),
so the whole schedule is built to keep that stream gapless and hide
everything else under it.

Dataflow (per core): L is processed in NQ quarter units, batch-inner, so
per (b, q) unit the sn DMA is one [128, QW] transfer per k-half on the SP
queue — nothing else ever issues on that queue, so the stream never stalls
on compute (the sn ring is kept shallow so the small auxiliary DMAs don't
queue behind prefetched transfers). f32r matmuls run at full PE rate;
tanh(+bias) is fused on ScalarE; the v-dot H-reduction runs as M=1
matmuls one chunk behind so the PE rarely waits on ScalarE. DVE evacuates
the v-dot PSUM rows into per-(group, quarter) staging tiles at partition
offsets 0/32/64/96 (the only offsets compute engines may write).

Softmax ("packed finish"): a tiny SBUF->SBUF DMA densifies each staging
tile's 4 sparse rows into a [128, 64] slice of the group's packed tile
(all 128 partitions useful), so exp(+negc shift, accumulating row sums)
costs ScalarE 1/32 of the naive width; per-batch totals come from one
[128,4]-indicator matmul, the reciprocal is broadcast back by a second
indicator matmul, one tensor_scalar_mul scales the whole group, and
per-quarter gather DMAs write straight to the output. Each quarter
processes group 1's batches first, so group 1's entire softmax chain
hides under the stream and only group 0's last exp/scale survives into
the tail. exp emission is deferred several chunks so its dependencies are
satisfied before ScalarE reaches it in queue order (the in-order wait
queues are only 4 deep).
"""

import os
import sys

import numpy as np

for _p in ("/opt/trn_rl_repo", "/root/.axon_site/_ro/trn_rl_repo"):
    if os.path.isdir(_p) and _p not in sys.path:
        sys.path.insert(0, _p)

import concourse.bass as bass  # noqa: E402
import concourse.tile as tile  # noqa: E402
from concourse import bacc, mybir  # noqa: E402
from concourse.bass_utils import run_bass_kernel_spmd  # noqa: E402

B, H, L = 64, 256, 8192
NCORES = 8
BL = B // NCORES  # batches per core
F32 = mybir.dt.float32
F32R = mybir.dt.float32r
PK_COLS = 4 * H + 2 * BL + 2  # packed replicated-param layout (see _emit_params)
NEG_COLS = 1 + 132  # negc constant | ind4 [128,4] | ind4T rows (see _emit_params)

CFG = {
    "qw": 2048,  # cols per (b, quarter) unit
    "ch": 1024,  # matmul/activation chunk width
    "seg": 512,  # v-dot segment width (one PSUM bank)
    "lag": 1,  # chunks the att stage trails the pre stage
    "sn_bufs": 4,
    "pre_bufs": 6,
    "ps_pre_bufs": 3,
    "ps_att_bufs": 2,
    "param_dma": "gpsimd",  # SWDGE queue: params lose the race for the first
    # DMA-engine slots, so the sn stream starts immediately
    "b_order": (4, 5, 6, 7, 0, 1, 2, 3),  # group 1 first within each quarter
    "sm_defer": 5,  # chunks between a group's last copy and its exp emission
    "exp_split": 4,  # pieces per (group, quarter) exp
    "exp_spread": 1,  # chunks between successive exp pieces
    "sums_on_dve": 0,  # row-sums via DVE reduce instead of ScalarE accum_out
    "copy_split": 0,  # (unusable: GPSIMD cannot read PSUM on real TRN2)
    "packed_finish": 1,  # densify staging via DMA before exp/scale (32x less ScalarE)
    "pack_w": 64,  # columns per partition in the packed tile
    "repack_lat": 2,  # chunks between repack DMA and its exp
    "exp_defer_extra": 12,  # extra defer for non-tail groups' exp (hides the
    # repack DMA's queueing behind prefetched sn transfers)
    "dma_half": 0,  # batch-outer loop with [128, 2*QW] sn DMAs
    "sn_bufs_half": 5,  # sn ring size in dma_half mode
    "first_split": 2,  # piecewise sn DMAs for the first N units (faster fill)
    "split_w": 512,  # piece width for first_split units
}


def _emit_params(tc: tile.TileContext, pools, w1t, negc):
    """Load replicated params, compute bias = W2 @ mc, init staging tiles.
    Emitted outside the timing loop."""
    nc = tc.nc
    singles, _, _, ps_pre, _ = pools
    prm = getattr(nc, CFG["param_dma"])
    QW = CFG["qw"]
    NQ = L // QW

    # all replicated params arrive in ONE packed [128, 1042] DMA (pk):
    # w1 k0 | w1 k1 | w2 k0 | w2 k1 | mct k0 | mct k1 | v k0 | v k1
    P = {}
    pk_sb = singles.tile([128, PK_COLS], F32R, tag="pk", name="pk_sb")
    prm.dma_start(out=pk_sb, in_=w1t)
    P["w1"] = [pk_sb[:, 0:H], pk_sb[:, H : 2 * H]]
    P["w2"] = [pk_sb[:, 2 * H : 3 * H], pk_sb[:, 3 * H : 4 * H]]
    P["mct"] = [
        pk_sb[:, 4 * H : 4 * H + BL],
        pk_sb[:, 4 * H + BL : 4 * H + 2 * BL],
    ]
    P["v"] = [
        pk_sb[:, 4 * H + 2 * BL : 4 * H + 2 * BL + 1],
        pk_sb[:, 4 * H + 2 * BL + 1 : 4 * H + 2 * BL + 2],
    ]
    P["negc"] = singles.tile([128, 1], F32, tag="negc", name="negc_sb")
    prm.dma_start(out=P["negc"], in_=negc[:, 0:1])
    if CFG["packed_finish"]:
        ind_sb = singles.tile([128, 132], F32, tag="ind", name="ind_sb")
        prm.dma_start(out=ind_sb, in_=negc[:, 1:133])
        P["ind4"] = ind_sb[:, 0:4]
        P["ind4T"] = ind_sb[0:4, 4:132]

    P["bias"] = []
    for m in range(2):
        bps = ps_pre.tile([128, BL], F32, tag="pspre", name=f"biasps_{m}")
        for k in range(2):
            nc.tensor.matmul(
                bps,
                lhsT=P["w2"][k][:, m * 128 : (m + 1) * 128],
                rhs=P["mct"][k],
                start=(k == 0),
                stop=(k == 1),
            )
        bsb = singles.tile([128, BL], F32, tag=f"bias_{m}", name=f"bias_{m}")
        nc.vector.tensor_copy(out=bsb, in_=bps)
        P["bias"].append(bsb)

    # persistent staging tiles (one per group x quarter); when the packed
    # finish is off they are zero-filled so the inactive partitions always
    # hold finite values for the in-place exp
    P["stg"] = {}
    for g in range(2):
        for q in range(NQ):
            st = singles.tile([128, QW], F32, tag=f"stg_{g}_{q}", name=f"stg_{g}_{q}")
            if not CFG["packed_finish"]:
                nc.gpsimd.memset(st[:], 0.0)
            P["stg"][(g, q)] = st
    if CFG["packed_finish"]:
        W = CFG["pack_w"]
        P["packed"] = {
            g: singles.tile([128, NQ * W], F32, tag=f"pkd_{g}", name=f"pkd_{g}")
            for g in range(2)
        }
        P["qacc"] = [
            singles.tile([128, NQ], F32, tag=f"qacc_{g}", name=f"qacc_{g}")
            for g in range(2)
        ]
    P["sums"] = [
        singles.tile([128, NQ * CFG["exp_split"]], F32, tag=f"sums_{g}", name=f"sums_{g}")
        for g in range(2)
    ]
    P["qsums"] = [
        singles.tile([128, NQ], F32, tag=f"qsums_{g}", name=f"qsums_{g}")
        for g in range(2)
    ]
    P["rec"] = [
        singles.tile([128, 1], F32, tag=f"rec_{g}", name=f"rec_{g}") for g in range(2)
    ]
    return P


def _emit_stream(tc: tile.TileContext, pools, P, sn, out, variant="full"):
    nc = tc.nc
    _, sn_pool, pre_pool, ps_pre, ps_att = pools
    QW = CFG["qw"]
    CH = CFG["ch"]
    SEG = CFG["seg"]
    LAG = CFG["lag"]
    NQ = L // QW
    NCH = QW // CH
    w1_sb, v_sb, bias_sb = P["w1"], P["v"], P["bias"]
    negc_sb = P["negc"]

    pending = []  # deferred att-stage closures (flushed at LAG depth)
    chunk_idx = [0]
    copy_ctr = [0]
    softmax_due = []  # (due chunk index, closure) — deferred so the exp's
    # copy dependencies are satisfied before ScalarE reaches it in order

    def flush_one():
        if pending:
            pending.pop(0)()
        chunk_idx[0] += 1
        while softmax_due and softmax_due[0][0] <= chunk_idx[0]:
            softmax_due.pop(0)[1]()

    tail_g = CFG["b_order"][-1] // 4  # the group whose softmax ends the kernel

    def exp_piece(g, q, e):
        ES = CFG["exp_split"]
        w = QW // ES
        stg = P["stg"][(g, q)]
        if CFG["sums_on_dve"]:
            # keep ScalarE lean: no accumulator read; DVE owns the row sums
            nc.scalar.activation(
                out=stg[:, e * w : (e + 1) * w],
                in_=stg[:, e * w : (e + 1) * w],
                func=mybir.ActivationFunctionType.Exp,
                bias=negc_sb,
            )
        else:
            nc.scalar.activation(
                out=stg[:, e * w : (e + 1) * w],
                in_=stg[:, e * w : (e + 1) * w],
                func=mybir.ActivationFunctionType.Exp,
                bias=negc_sb,
                accum_out=P["sums"][g][:, q * ES + e : q * ES + e + 1],
            )

    def quarter_sum(g, q):
        nc.vector.reduce_sum(
            out=P["qsums"][g][:, q : q + 1],
            in_=P["stg"][(g, q)],
            axis=mybir.AxisListType.X,
        )

    W = CFG["pack_w"]

    def repack(g, q):
        # densify: staging rows 0/32/64/96 x QW cols -> [128, W] packed slice
        # (flat element order matches: p*W+e == b*QW + chunk*W + e)
        nc.gpsimd.dma_start(
            out=P["packed"][g][:, q * W : (q + 1) * W],
            in_=P["stg"][(g, q)][0:97:32, :],
        )

    def exp_packed(g, q):
        pkd = P["packed"][g][:, q * W : (q + 1) * W]
        nc.scalar.activation(
            out=pkd,
            in_=pkd,
            func=mybir.ActivationFunctionType.Exp,
            bias=negc_sb,
            accum_out=P["qacc"][g][:, q : q + 1],
        )

    def finish_packed(g):
        """per-batch totals via indicator matmuls, reciprocal broadcast,
        scale+store of the packed tiles."""
        t128 = pools[0].tile([128, 1], F32, tag=f"t128_{g}", name=f"t128_{g}")
        nc.vector.reduce_sum(out=t128, in_=P["qacc"][g], axis=mybir.AxisListType.X)
        ps4 = ps_att.tile([4, 1], F32, tag="psatt", name=f"ps4_{g}")
        nc.tensor.matmul(ps4, lhsT=P["ind4"], rhs=t128, start=True, stop=True)
        rec4 = pools[0].tile([4, 1], F32, tag=f"rec4_{g}", name=f"rec4_{g}")
        nc.vector.reciprocal(out=rec4, in_=ps4)
        psr = ps_att.tile([128, 1], F32, tag="psatt", name=f"psr_{g}")
        nc.tensor.matmul(psr, lhsT=P["ind4T"], rhs=rec4, start=True, stop=True)
        recp = pools[0].tile([128, 1], F32, tag=f"recp_{g}", name=f"recp_{g}")
        nc.vector.tensor_copy(out=recp, in_=psr)
        pkd = P["packed"][g]
        nc.vector.tensor_scalar_mul(out=pkd, in0=pkd, scalar1=recp)
        for qq in range(NQ):
            if g == tail_g:
                eng = nc.sync if qq % 2 == 0 else nc.scalar
            else:
                eng = nc.gpsimd
            eng.dma_start(
                out=out[4 * g : 4 * g + 4, qq],
                in_=pkd[:, qq * W : (qq + 1) * W],
            )

    def group_finish(g):
        """total sum, reciprocal, scale+store for all quarters of group g."""
        tot = pools[0].tile([128, 1], F32, tag=f"tot_{g}", name=f"tot_{g}")
        src = P["qsums"][g] if CFG["sums_on_dve"] else P["sums"][g]
        nc.vector.reduce_sum(out=tot, in_=src, axis=mybir.AxisListType.X)
        nc.vector.reciprocal(out=P["rec"][g], in_=tot)
        for qq in range(NQ):
            sq = P["stg"][(g, qq)]
            if qq % 2 == 0:
                nc.vector.tensor_scalar_mul(out=sq, in0=sq, scalar1=P["rec"][g])
            else:
                nc.scalar.activation(
                    out=sq,
                    in_=sq,
                    func=mybir.ActivationFunctionType.Copy,
                    scale=P["rec"][g],
                )
            # never on the SP queue mid-stream — the SP DMA ring is FIFO, so a
            # parked output DMA would block later sn transfers. For the tail
            # group the stream is over; alternate queues so DGE gens overlap.
            if g == tail_g:
                eng = nc.sync if qq % 2 == 0 else nc.scalar
            else:
                eng = nc.gpsimd
            eng.dma_start(
                out=out[4 * g : 4 * g + 4, qq * QW : (qq + 1) * QW],
                in_=sq[0:97:32, :],
            )

    def emit_unit(bi, b, q, snt):
        if True:
            g = b // 4
            po = (b % 4) * 32  # staging partition offset
            stg = P["stg"][(g, q)]
            group_done = bi % 4 == 3  # this b completes its group's quarter
            for c in range(NCH):
                col0 = c * CH
                pre_sbs = []
                for m in range(2):
                    pps = ps_pre.tile([128, CH], F32, tag="pspre", name=f"pps_{q}_{b}_{c}_{m}")
                    for s in range(CH // 512):
                        for k in range(2):
                            nc.tensor.matmul(
                                pps[:, s * 512 : (s + 1) * 512],
                                lhsT=w1_sb[k][:, m * 128 : (m + 1) * 128],
                                rhs=snt[k][:, col0 + s * 512 : col0 + (s + 1) * 512],
                                start=(k == 0),
                                stop=(k == 1),
                            )
                    if variant == "mm_only":
                        continue
                    psb = pre_pool.tile([128, CH], F32R, tag="pre", name=f"pre_{q}_{b}_{c}_{m}")
                    nc.scalar.activation(
                        out=psb,
                        in_=pps,
                        func=mybir.ActivationFunctionType.Tanh,
                        bias=bias_sb[m][:, b : b + 1],
                    )
                    pre_sbs.append(psb)
                if variant in ("mm_only", "pre_only"):
                    continue

                last_chunk = c == NCH - 1

                def att_stage(
                    q=q, b=b, g=g, po=po, c=c, col0=col0,
                    stg=stg, pre_sbs=pre_sbs,
                    fin=(last_chunk and group_done and variant == "full"),
                ):
                    for s in range(CH // SEG):
                        aps = ps_att.tile([1, SEG], F32, tag="psatt", name=f"att_{q}_{b}_{c}_{s}")
                        for sp in range(SEG // 512):
                            for m in range(2):
                                nc.tensor.matmul(
                                    aps[:, sp * 512 : (sp + 1) * 512],
                                    lhsT=v_sb[m],
                                    rhs=pre_sbs[m][:, s * SEG + sp * 512 : s * SEG + (sp + 1) * 512],
                                    start=(m == 0),
                                    stop=(m == 1),
                                )
                        copy_ctr[0] += 1
                        ceng = (
                            nc.gpsimd
                            if CFG["copy_split"] and copy_ctr[0] % 2
                            else nc.vector
                        )
                        ceng.tensor_copy(
                            out=stg[po : po + 1, col0 + s * SEG : col0 + (s + 1) * SEG],
                            in_=aps,
                        )
                    if fin:
                        due = chunk_idx[0] + CFG["sm_defer"]
                        if CFG["packed_finish"]:
                            softmax_due.append(
                                (due, lambda g=g, q=q: repack(g, q))
                            )
                            after_exp = due + CFG["repack_lat"]
                            if g != tail_g:
                                after_exp += CFG["exp_defer_extra"]
                            softmax_due.append(
                                (after_exp, lambda g=g, q=q: exp_packed(g, q))
                            )
                            if q == NQ - 1 and not CFG.get("skip_finish"):
                                softmax_due.append(
                                    (after_exp + 1, lambda g=g: finish_packed(g))
                                )
                        else:
                            for e in range(CFG["exp_split"]):
                                softmax_due.append(
                                    (due + e * CFG["exp_spread"],
                                     lambda g=g, q=q, e=e: exp_piece(g, q, e))
                                )
                            after_exp = due + CFG["exp_split"] * CFG["exp_spread"]
                            if CFG["sums_on_dve"]:
                                softmax_due.append(
                                    (after_exp, lambda g=g, q=q: quarter_sum(g, q))
                                )
                                after_exp += 1
                            if q == NQ - 1:
                                softmax_due.append(
                                    (after_exp, lambda g=g: group_finish(g))
                                )

                pending.append(att_stage)
                softmax_due.sort(key=lambda x: x[0])
                if len(pending) > LAG:
                    flush_one()

    if CFG["dma_half"]:
        # batch-outer: one [128, 2*QW] DMA per (b, k, L-half) — half as many,
        # twice-larger descriptors; units consume tile views
        for bi, b in enumerate(CFG["b_order"]):
            snt_half = []
            for q in range(NQ):
                if q % 2 == 0:
                    h = q // 2
                    snt_half = []
                    for k in range(2):
                        t = sn_pool.tile(
                            [128, 2 * QW], F32R, tag="sn", name=f"sn_{b}_{h}_{k}"
                        )
                        nc.sync.dma_start(
                            out=t,
                            in_=sn[
                                b,
                                k * 128 : (k + 1) * 128,
                                h * 2 * QW : (h + 1) * 2 * QW,
                            ],
                        )
                        snt_half.append(t)
                if variant == "dma_only":
                    continue
                snt = [t[:, (q % 2) * QW : (q % 2 + 1) * QW] for t in snt_half]
                emit_unit(bi, b, q, snt)
    else:
        unit_no = 0
        for q in range(NQ):
            for bi, b in enumerate(CFG["b_order"]):
                split = unit_no < CFG["first_split"]
                snt = [
                    sn_pool.tile([128, QW], F32R, tag="sn", name=f"sn_{q}_{b}_{k}")
                    for k in range(2)
                ]
                if split:
                    # piecewise, k-interleaved, so the PE can start on the
                    # first columns while the rest is still in flight
                    SW = CFG["split_w"]
                    for o in range(0, QW, SW):
                        for k in range(2):
                            nc.sync.dma_start(
                                out=snt[k][:, o : o + SW],
                                in_=sn[
                                    b,
                                    k * 128 : (k + 1) * 128,
                                    q * QW + o : q * QW + o + SW,
                                ],
                            )
                else:
                    for k in range(2):
                        nc.sync.dma_start(
                            out=snt[k],
                            in_=sn[b, k * 128 : (k + 1) * 128, q * QW : (q + 1) * QW],
                        )
                unit_no += 1
                if variant == "dma_only":
                    continue
                emit_unit(bi, b, q, snt)
    while pending or softmax_due:
        flush_one()


def build_module(variant="full", loop_n=None):
    from contextlib import ExitStack

    nc = bacc.Bacc(
        "TRN2",
        debug=False,
        enable_asserts=False,
        target_bir_lowering=False,
    )
    sn = nc.dram_tensor("sn", [BL, H, L], F32R, kind="ExternalInput").ap()
    pk = nc.dram_tensor("pk", [128, PK_COLS], F32R, kind="ExternalInput").ap()
    negc = nc.dram_tensor("negc", [128, NEG_COLS], F32, kind="ExternalInput").ap()
    if CFG["packed_finish"]:
        # same bytes as [BL, L]: row-major [b][q][chunk][W] == [b][q*2048+...]
        NQ = L // CFG["qw"]
        W = CFG["pack_w"]
        out = nc.dram_tensor(
            "out", [BL, NQ, CFG["qw"] // W, W], F32, kind="ExternalOutput"
        ).ap()
    else:
        out = nc.dram_tensor("out", [BL, L], F32, kind="ExternalOutput").ap()
    with tile.TileContext(nc) as tc:
        with ExitStack() as ctx:
            sn_bufs = CFG["sn_bufs_half"] if CFG["dma_half"] else CFG["sn_bufs"]
            pools = (
                ctx.enter_context(tc.tile_pool(name="singles", bufs=1)),
                ctx.enter_context(tc.tile_pool(name="snp", bufs=sn_bufs)),
                ctx.enter_context(tc.tile_pool(name="prep", bufs=CFG["pre_bufs"])),
                ctx.enter_context(
                    tc.tile_pool(name="pspre", bufs=CFG["ps_pre_bufs"], space="PSUM")
                ),
                ctx.enter_context(
                    tc.tile_pool(name="psatt", bufs=CFG["ps_att_bufs"], space="PSUM")
                ),
            )
            P = _emit_params(tc, pools, pk, negc)
            if loop_n is not None:
                with tc.For_i(
                    0,
                    loop_n,
                    1,
                    hint_engines=(
                        mybir.EngineType.PE,
                        mybir.EngineType.Activation,
                        mybir.EngineType.DVE,
                        mybir.EngineType.SP,
                        mybir.EngineType.Pool,
                    ),
                ):
                    _emit_stream(tc, pools, P, sn, out, variant=variant)
            else:
                _emit_stream(tc, pools, P, sn, out, variant=variant)
    nc.compile()
    return nc


_NC = None


def _get_module():
    global _NC
    if _NC is None:
        _NC = build_module()
    return _NC


def make_in_maps(mc_hidden, sn_hidden, v, W):
    """Shard FULL inputs into per-core in_maps (host-side, cheap)."""
    w0 = np.asarray(W, dtype=np.float32)[0]  # [H, 2H]
    w1t = np.ascontiguousarray(w0[:, :H].T)  # [H(k), H(h)]
    w2t = np.ascontiguousarray(w0[:, H:].T)  # [H(k), H(h)]
    vcol = np.ascontiguousarray(np.asarray(v, dtype=np.float32)[0, 0][:, None])
    # upper bound on |attns| = |v . tanh(...)| <= ||v||_1; softmax is invariant
    # to the shift and exp(x - c) stays in fp32 range. Columns 1:133 carry the
    # batch-indicator blocks for the packed softmax finish.
    negc = np.zeros((128, NEG_COLS), dtype=np.float32)
    negc[:, 0] = -np.abs(vcol).sum()
    p_idx = np.arange(128)
    negc[:, 1:5] = (p_idx[:, None] // 32 == np.arange(4)).astype(np.float32)
    negc[0:4, 5:133] = (np.arange(4)[:, None] == p_idx[None, :] // 32).astype(
        np.float32
    )
    mc = np.asarray(mc_hidden, dtype=np.float32)
    sn = np.asarray(sn_hidden, dtype=np.float32)
    in_maps = []
    for c in range(NCORES):
        sl = slice(c * BL, (c + 1) * BL)
        mct = mc[sl].T  # [H, BL]
        pk = np.zeros((128, PK_COLS), dtype=np.float32)
        pk[:, 0:H] = w1t[0:128]
        pk[:, H : 2 * H] = w1t[128:256]
        pk[:, 2 * H : 3 * H] = w2t[0:128]
        pk[:, 3 * H : 4 * H] = w2t[128:256]
        pk[:, 4 * H : 4 * H + BL] = mct[0:128]
        pk[:, 4 * H + BL : 4 * H + 2 * BL] = mct[128:256]
        pk[:, 4 * H + 2 * BL : 4 * H + 2 * BL + 1] = vcol[0:128]
        pk[:, 4 * H + 2 * BL + 1 : 4 * H + 2 * BL + 2] = vcol[128:256]
        in_maps.append(
            {
                "sn": np.ascontiguousarray(sn[sl]),
                "pk": pk,
                "negc": negc,
            }
        )
    return in_maps


def run(mc_hidden, sn_hidden, v, W, trace=False):
    nc = _get_module()
    in_maps = make_in_maps(mc_hidden, sn_hidden, v, W)
    res = run_bass_kernel_spmd(nc, in_maps, core_ids=list(range(NCORES)), trace=False)
    full = np.concatenate(
        [np.asarray(r["out"]).reshape(BL, L) for r in res.results], axis=0
    )
    return full[:, None, :].astype(np.float32), res


def kernel(mc_hidden, sn_hidden, v, W):
    out, _ = run(mc_hidden, sn_hidden, v, W, trace=False)
    return out
